# revision 16
# baseline (speedup 1.0000x reference)
"""Bidirectional Mamba on 8 Trainium2 NeuronCores.

Sharding: 8 cores = (2 directions) x (4 batch elements); each core runs one
full Mamba block on its (L=1024, DM=512) sequence. The backward direction is
handled by flipping the sequence on the host before/after, so all cores run
the identical SPMD program with different data.

Layout: channels d on partitions, time t on the free dim; the d=512 channels
form 4 chunks of 128, paired into (128, 2048) tiles. The selective scan
(tensor_tensor_scan, ~2.5 ns/element measured) is the hard DVE floor, so
everything else is pushed off the DVE:
  - Act: dA = exp(dt*A_n) (bf16 out, persistent zero column at the pair
    boundary restarts the scan for the 2nd chunk)
  - DVE: the 32 scans + dBx = u*B_n (broadcast bf16 multiply, ~0.6us)
  - Pool (GPSIMD): hC = h*C_n and the gating multiplies
  - PE: GEMMs (A f32r, B/C/D bf16, conv fp32 diagonal matmuls), y += hC
    via bf16 identity matmuls into PSUM
B_n/C_n come from one partition-broadcast DMA per (h, n) out of a bf16 DRAM
copy of dbc. The chunk-pair loop runs h-outer so y needs only 4 PSUM banks,
leaving 4 for the next pass's GEMMs: timing builds unroll two passes per
For_i iteration with ping-ponged activation tiles so pass r+1's PE/Act head
overlaps pass r's DVE-bound scan phase.
"""
import contextlib

import numpy as np

import concourse.bacc as bacc
import concourse.tile as tile
import concourse.mybir as mybir
from concourse.bass_utils import run_bass_kernel_spmd

F32 = mybir.dt.float32
F32R = mybir.dt.float32r
BF16 = mybir.dt.bfloat16
AF = mybir.ActivationFunctionType
OP = mybir.AluOpType

DM = 512
DI = 512
L = 1024
N = 16
K = 4
R = 32
P = 128
NCH = DI // P          # 4 d-chunks
W = 2 * L              # wide tile free size (chunk pair)
TB = 512               # matmul out block (1 PSUM bank)
NTB = L // TB          # 2
CPAD = K - 1
N_CORES = 8

# scan-phase multiply placement: dBx on DVE; hC on Pool except these n
HC_ON_DVE = set()


def _mm(nc, out, lhsT, rhs, start, stop):
    nc.tensor.matmul(out, lhsT=lhsT, rhs=rhs, start=start, stop=stop,
                     skip_group_check=True)


def emit_mamba(tc, io, dbc_dram, weights, tset, pools):
    """One full mamba pass. `weights` holds the persistent weight tiles,
    `tset` the pass-parity activation tiles (ping-ponged across passes),
    `pools` the shared pools (persistent addresses so consecutive passes
    never alias each other's memory through stack reuse)."""
    nc = tc.nc
    f32 = F32
    (small_sb, Wx_sb, Wdt_sb, Wout_sb, Wdiag_sb, ident_sb) = weights
    (gin, psAB, pscv, spl, sp, bcp, hcp, tmpp, psy, osb) = pools
    (zs_sb, xs_sb, dt_sb, u_sb, yz_sb, dA_sb, dtin_sb, dbcb_sb) = tset

    SP_W = K + 1 + 1 + N + 1
    o_wc, o_bc, o_bdt, o_a, o_d = 0, K, K + 1, K + 2, K + 2 + N

    def bconv(dc):
        return small_sb[:, dc, o_bc:o_bc + 1]

    def bdt(dc):
        return small_sb[:, dc, o_bdt:o_bdt + 1]

    def A_sc(dc):
        return small_sb[:, dc, o_a:o_a + N]

    def Dv(dc):
        return small_sb[:, dc, o_d:o_d + 1]

    def wide(arr, dc, lo=0, hi=L):
        return arr[dc // 2][:, (dc % 2) * L + lo : (dc % 2) * L + hi]

    # ---- head: GEMM A (xc half), conv, GEMM B, GEMM C, GEMM A (z half) ----
    if True:
        W_in_sb = gin.tile([P, NCH, 2 * DI], F32R, tag="Wi", name="Wi")
        xT_sb = gin.tile([P, NCH, L], F32R, tag="xT", name="xT")
        xcp_sb = [
            gin.tile([P, CPAD + L], f32, tag=f"xcp{i}", name=f"xcp{i}")
            for i in range(NCH)
        ]
        nc.sync.dma_start(W_in_sb[:], io["W_in"][:, :, :])
        nc.sync.dma_start(xT_sb[:], io["xT"][:, :, :])
        for i in range(NCH):
            nc.vector.memset(xcp_sb[i][:, 0:CPAD], 0.0)

        def gemmA(cb_range):
            for cb in cb_range:
                for tb in range(NTB):
                    ps = psAB.tile([P, TB], f32, tag="psAB", name="psAB")
                    for mk in range(NCH):
                        _mm(
                            nc, ps[:],
                            W_in_sb[:, mk, cb * P : (cb + 1) * P],
                            xT_sb[:, mk, tb * TB : (tb + 1) * TB],
                            start=(mk == 0), stop=(mk == NCH - 1),
                        )
                    lo, hi = tb * TB, (tb + 1) * TB
                    if cb < NCH:
                        nc.scalar.activation(
                            xcp_sb[cb][:, CPAD + lo : CPAD + hi], ps[:],
                            AF.Copy,
                        )
                    else:
                        nc.scalar.activation(
                            wide(zs_sb, cb - NCH, lo, hi), ps[:], AF.Silu
                        )

        gemmA(range(NCH))  # xc half

        # causal depthwise conv on PE: fp32 diagonal matmuls, PSUM accumulate
        if True:
            for dc in range(NCH):
                cps = pscv.tile([P, L], f32, tag="cps", name="cps")
                for tb in range(NTB):
                    osl = slice(tb * TB, (tb + 1) * TB)
                    for s in range(K):
                        k = K - 1 - s
                        dg = Wdiag_sb[:, (dc * K + k) * P : (dc * K + k + 1) * P]
                        _mm(nc, cps[:, osl], dg,
                            xcp_sb[dc][:, CPAD - s + tb * TB
                                       : CPAD - s + tb * TB + TB],
                            start=(s == 0), stop=(s == K - 1))
                nc.scalar.activation(
                    wide(xs_sb, dc), cps[:, 0:L], AF.Silu,
                    bias=bconv(dc)[:, 0:1],
                )

        # GEMM B (bf16): dbc rows = [dt_in(R) | B(N) | C(N)]
        for tb in range(NTB):
            ps = psAB.tile([R + 2 * N, TB], f32, tag="psAB", name="psAB")
            for dc in range(NCH):
                _mm(
                    nc, ps[:], Wx_sb[:, dc, :],
                    wide(xs_sb, dc, tb * TB, (tb + 1) * TB),
                    start=(dc == 0), stop=(dc == NCH - 1),
                )
            sl = slice(tb * TB, (tb + 1) * TB)
            nc.scalar.activation(dtin_sb[:, sl], ps[0:R, :], AF.Copy)
            nc.scalar.activation(dbcb_sb[:, sl], ps[R : R + 2 * N, :], AF.Copy)
        nc.sync.dma_start(dbc_dram[:, :], dbcb_sb[:])

        # GEMM C (bf16) + softplus -> dt
        if True:
            for dc in range(NCH):
                for tb in range(NTB):
                    ps = psAB.tile([P, TB], f32, tag="psAB", name="psAB")
                    _mm(
                        nc, ps[:], Wdt_sb[:, dc * P : (dc + 1) * P],
                        dtin_sb[:, tb * TB : (tb + 1) * TB],
                        start=True, stop=True,
                    )
                    et = spl.tile([P, TB], f32, tag="et", name="et")
                    nc.scalar.activation(et[:], ps[:], AF.Exp,
                                         bias=bdt(dc)[:, 0:1])
                    nc.scalar.activation(
                        wide(dt_sb, dc, tb * TB, (tb + 1) * TB),
                        et[:], AF.Ln, bias=1.0,
                    )

        gemmA(range(NCH, 2 * NCH))  # z half (needed only at gating)

    # u = dt * xs (bf16) on Pool
    for h in range(2):
        nc.gpsimd.tensor_tensor(u_sb[h][:], dt_sb[h][:], xs_sb[h][:],
                                op=OP.mult)

    # ---- selective scan: h outer (y needs only 4 PSUM banks) ----
    if True:
        for h in range(2):
            chunks = (2 * h, 2 * h + 1)
            y_ps = psy.tile([P, W], f32, tag="y", name="y")
            for n in range(N):
                BCb = bcp.tile([P, 2, L], BF16, tag="BCb", name="BCb")
                nc.sync.dma_start(
                    BCb[:],
                    dbc_dram[n : N + n + 1 : N, :].partition_broadcast(P),
                )
                Bb, Cb = BCb[:, 0, :], BCb[:, 1, :]
                dA = dA_sb[n % 2]
                nc.scalar.activation(
                    dA[:, 0:L], wide(dt_sb, chunks[0]), AF.Exp,
                    scale=A_sc(chunks[0])[:, n : n + 1],
                )
                nc.scalar.activation(
                    dA[:, L + 1 : W], wide(dt_sb, chunks[1], 1, L), AF.Exp,
                    scale=A_sc(chunks[1])[:, n : n + 1],
                )

                dBx = sp.tile([P, W], BF16, tag="dBx", name="dBx")
                nc.vector.tensor_tensor(
                    dBx[:].rearrange("p (r f) -> p r f", r=2),
                    u_sb[h][:].rearrange("p (r f) -> p r f", r=2),
                    Bb.unsqueeze(1).broadcast_to((P, 2, L)),
                    op=OP.mult,
                )
                # scan in place: h overwrites dBx (fp32 internal state)
                nc.vector.tensor_tensor_scan(
                    dBx[:], dA[:], dBx[:], 0.0, op0=OP.mult, op1=OP.add
                )
                hC = hcp.tile([P, W], BF16, tag="hC", name="hC")
                hc_eng = nc.vector if n in HC_ON_DVE else nc.gpsimd
                hc_eng.tensor_tensor(
                    hC[:].rearrange("p (r f) -> p r f", r=2),
                    dBx[:].rearrange("p (r f) -> p r f", r=2),
                    Cb.unsqueeze(1).broadcast_to((P, 2, L)),
                    op=OP.mult,
                )
                for tb in range(W // TB):
                    tsl = slice(tb * TB, (tb + 1) * TB)
                    _mm(nc, y_ps[:, tsl], ident_sb[:], hC[:, tsl],
                        start=(n == 0), stop=(n == N - 1))

            # yz = (y + D*xs) * silu(z): stt on DVE, gate mult on Pool
            tmp = tmpp.tile([P, W], BF16, tag="tmp", name="tmp")
            for dc in chunks:
                q = (dc % 2) * L
                nc.vector.scalar_tensor_tensor(
                    out=tmp[:, q : q + L],
                    in0=wide(xs_sb, dc),
                    scalar=Dv(dc)[:, 0:1],
                    in1=y_ps[:, q : q + L],
                    op0=OP.mult,
                    op1=OP.add,
                )
            nc.gpsimd.tensor_tensor(yz_sb[h][:], tmp[:], zs_sb[h][:],
                                    op=OP.mult)

    # ---- GEMM D (bf16), PSUM shared with the head pool ----
    if True:
        for mb in range(DM // P):
            ot = osb.tile([P, L], f32, tag="ot", name="ot")
            for tb in range(NTB):
                ps = psAB.tile([P, TB], f32, tag="psAB", name="psAB")
                for dc in range(NCH):
                    _mm(
                        nc, ps[:],
                        Wout_sb[:, dc, mb * P : (mb + 1) * P],
                        wide(yz_sb, dc, tb * TB, (tb + 1) * TB),
                        start=(dc == 0), stop=(dc == NCH - 1),
                    )
                nc.scalar.activation(
                    ot[:, tb * TB : (tb + 1) * TB], ps[:], AF.Copy
                )
            nc.sync.dma_start(io["outT"][mb * P : (mb + 1) * P, :], ot[:])


def emit_all(tc, io, n_tsets):
    nc = tc.nc
    f32 = F32
    SP_W = K + 1 + 1 + N + 1
    with contextlib.ExitStack() as ctx:
        per = ctx.enter_context(tc.tile_pool(name="per", bufs=1))

        def ptile(tag, shape, dtype=f32):
            return per.tile(shape, dtype, tag=tag, name=tag)

        small_sb = ptile("small", [P, NCH, SP_W])
        Wx_sb = ptile("Wx", [P, NCH, R + 2 * N], BF16)
        Wdt_sb = ptile("Wdt", [R, DI], BF16)
        Wout_sb = ptile("Wout", [P, NCH, DM], BF16)
        Wdiag_sb = ptile("Wdiag", [P, NCH * K * P])
        ident_sb = ptile("ident", [P, P], BF16)
        weights = (small_sb, Wx_sb, Wdt_sb, Wout_sb, Wdiag_sb, ident_sb)

        tsets = []
        for ts in range(n_tsets):
            zs = [ptile(f"zs{ts}{i}", [P, W], BF16) for i in range(2)]
            xs = [ptile(f"xs{ts}{i}", [P, W], BF16) for i in range(2)]
            dt = [ptile(f"dt{ts}{i}", [P, W], BF16) for i in range(2)]
            u = [ptile(f"u{ts}{i}", [P, W], BF16) for i in range(2)]
            yz = [ptile(f"yz{ts}{i}", [P, W], BF16) for i in range(2)]
            dA = [ptile(f"dA{ts}{pr}", [P, W], BF16) for pr in range(2)]
            dtin = ptile(f"dtin{ts}", [R, L], BF16)
            dbcb = ptile(f"dbcb{ts}", [2 * N, L], BF16)
            tsets.append((zs, xs, dt, u, yz, dA, dtin, dbcb))
            for pr in range(2):
                nc.vector.memset(dA[pr][:, L : L + 1], 0.0)

        def load_weights():
            nc.sync.dma_start(small_sb[:], io["small"][:, :, :])
            nc.sync.dma_start(Wx_sb[:], io["Wx"][:, :, :])
            nc.sync.dma_start(Wdt_sb[:], io["Wdt"][:, :])
            nc.sync.dma_start(Wout_sb[:], io["W_out"][:, :, :])
            nc.sync.dma_start(Wdiag_sb[:], io["Wdiag"][:, :])
            nc.sync.dma_start(ident_sb[:], io["ident"][:, :])

        gin = ctx.enter_context(tc.tile_pool(name="gin", bufs=1))
        psAB = ctx.enter_context(tc.tile_pool(name="psAB", bufs=2, space="PSUM"))
        pscv = ctx.enter_context(tc.tile_pool(name="pscv", bufs=1, space="PSUM"))
        spl = ctx.enter_context(tc.tile_pool(name="spl", bufs=2))
        sp = ctx.enter_context(tc.tile_pool(name="scan", bufs=2))
        bcp = ctx.enter_context(tc.tile_pool(name="bcp", bufs=2))
        hcp = ctx.enter_context(tc.tile_pool(name="hcp", bufs=2))
        tmpp = ctx.enter_context(tc.tile_pool(name="tmp", bufs=2))
        psy = ctx.enter_context(tc.tile_pool(name="psy", bufs=1, space="PSUM"))
        osb = ctx.enter_context(tc.tile_pool(name="osb", bufs=1))
        pools = (gin, psAB, pscv, spl, sp, bcp, hcp, tmpp, psy, osb)
        yield weights, tsets, load_weights, pools


def build(reps=1):
    nc = bacc.Bacc(
        "TRN2",
        target_bir_lowering=False,
        debug=False,
        enable_asserts=False,
        num_devices=N_CORES,
    )
    SP_W = K + 1 + 1 + N + 1
    io = {
        "xT": nc.dram_tensor("xT", (P, NCH, L), F32R, kind="ExternalInput").ap(),
        "W_in": nc.dram_tensor("W_in", (P, NCH, 2 * DI), F32R, kind="ExternalInput").ap(),
        "small": nc.dram_tensor("small", (P, NCH, SP_W), F32, kind="ExternalInput").ap(),
        "Wx": nc.dram_tensor("Wx", (P, NCH, R + 2 * N), BF16, kind="ExternalInput").ap(),
        "Wdt": nc.dram_tensor("Wdt", (R, DI), BF16, kind="ExternalInput").ap(),
        "W_out": nc.dram_tensor("W_out", (P, NCH, DM), BF16, kind="ExternalInput").ap(),
        "Wdiag": nc.dram_tensor("Wdiag", (P, NCH * K * P), F32, kind="ExternalInput").ap(),
        "ident": nc.dram_tensor("ident", (P, P), BF16, kind="ExternalInput").ap(),
        "outT": nc.dram_tensor("outT", (DM, L), F32, kind="ExternalOutput").ap(),
        "dbc_bf0": nc.dram_tensor("dbc_bf0", (2 * N, L), BF16).ap(),
        "dbc_bf1": nc.dram_tensor("dbc_bf1", (2 * N, L), BF16).ap(),
    }
    n_tsets = 1 if reps == 1 else 2
    with tile.TileContext(nc) as tc:
        gen = emit_all(tc, io, n_tsets)
        weights, tsets, load_weights, pools = next(gen)
        load_weights()
        if reps == 1:
            emit_mamba(tc, io, io["dbc_bf0"], weights, tsets[0], pools)
        else:
            assert reps % 2 == 0
            with tc.For_i(0, reps // 2, 1):
                emit_mamba(tc, io, io["dbc_bf0"], weights, tsets[0], pools)
                emit_mamba(tc, io, io["dbc_bf1"], weights, tsets[1], pools)
        # close the ExitStack inside emit_all
        try:
            next(gen)
        except StopIteration:
            pass
    nc.compile()
    return nc


_NC_CACHE = {}


def _get_nc(reps=1):
    if reps not in _NC_CACHE:
        _NC_CACHE[reps] = build(reps)
    return _NC_CACHE[reps]


def _chunked(a):
    """(DI, X) -> (P, NCH, X) with chunk dc = rows dc*P:(dc+1)*P."""
    return np.ascontiguousarray(
        np.asarray(a, np.float32).reshape(NCH, P, -1).transpose(1, 0, 2)
    )


def make_in_maps(inputs):
    import ml_dtypes

    x = np.asarray(inputs["x"], np.float32)
    SP_W = K + 1 + 1 + N + 1
    ident_bf = np.eye(P, dtype=ml_dtypes.bfloat16)
    in_maps = []
    for c in range(N_CORES):
        b = c % 4
        sfx = "f" if c < 4 else "b"
        xb = x[b] if c < 4 else x[b][::-1]

        def g(name):
            return np.asarray(inputs[f"{name}_{sfx}"], np.float32)

        Wc_ = g("W_conv")                      # (DI, K)
        small = np.concatenate(
            [
                Wc_,
                g("b_conv").reshape(DI, 1),
                g("b_dt").reshape(DI, 1),
                -np.exp(g("A_log")),
                g("D").reshape(DI, 1),
            ],
            axis=1,
        )
        assert small.shape == (DI, SP_W)
        wdiag = np.zeros((P, NCH * K * P), np.float32)
        for dc in range(NCH):
            for k in range(K):
                blk = slice((dc * K + k) * P, (dc * K + k + 1) * P)
                np.fill_diagonal(wdiag[:, blk], Wc_[dc * P : (dc + 1) * P, k])

        in_maps.append(
            {
                "xT": _chunked(xb.T),
                "W_in": _chunked(g("W_in")),
                "small": _chunked(small),
                "Wx": _chunked(g("W_xproj")).astype(ml_dtypes.bfloat16),
                "Wdt": np.ascontiguousarray(g("W_dt")).astype(ml_dtypes.bfloat16),
                "W_out": _chunked(g("W_out")).astype(ml_dtypes.bfloat16),
                "Wdiag": wdiag,
                "ident": ident_bf,
            }
        )
    return in_maps


def assemble_output(results):
    out = np.empty((4, L, DM), np.float32)
    for b in range(4):
        of = results[b]["outT"].T
        ob = results[4 + b]["outT"].T[::-1]
        out[b] = of + ob
    return out


def kernel(**inputs):
    nc = _get_nc()
    in_maps = make_in_maps(inputs)
    res = run_bass_kernel_spmd(nc, in_maps, core_ids=list(range(N_CORES)))
    return assemble_output(res.results)
